# revision 5
# baseline (speedup 1.0000x reference)
"""Trainium2 Bass kernel for nn_BasicBlock_72928544686679.

Computation (see the reference):
    s  = sign(x)                       # binary activation forward value
    bw = sign(w)                       # binary weights  (w in [0, 0.001) -> ~all ones)
    y' = conv2d(s, bw, pad=1)          # saturating conv: clip at +-2^31 never
                                       # binds (|acc| <= 2304), so it's a plain conv.
    y  = y' * scale[c],  scale = mean|w| over (cin,kh,kw)
    out = BN_trainmode(y) * gamma + beta + x

Sharding: data-parallel over batch B=16 -> 2 images per core on 8 cores.
BN statistics need the full batch, so each core computes per-channel partial
sums (sum y', sum y'^2) and a 2 KiB AllReduce combines them; scaling/eps are
folded analytically into a per-channel affine (A, B) applied in a second pass.

All sign values are exactly representable in bf16 and PSUM accumulates fp32,
so the conv results are exact integers == the reference f32 conv.
"""

import numpy as np

B = 16
NCORES = 8
IMG = 2            # images per core
C = 256            # Cin == Cout
H = W = 28
P = 128
CT = 2             # Cout tiles of 128
CIN_T = 2          # Cin tiles of 128
KPOS = 9           # 3x3 positions
HP, WP = 30, 32    # padded image rows / row stride (28+2 pad, 32 for alignment)
LH = 14            # output rows per L-half
N_HALF = LH * W    # 392, matmul free dim (one PSUM bank)
EPS = 1e-5
NLOC = float(IMG * H * W)   # 1568  elements per channel per core
NTOT = float(B * H * W)     # 12544 elements per channel globally

_NC_CACHE = {}
LAST_RESULTS = None  # BassKernelResults of the most recent run (for profiling)


def _build_nc():
    import concourse.mybir as mybir
    import concourse.tile as tile
    from concourse import bacc

    f32 = mybir.dt.float32
    bf16 = mybir.dt.bfloat16
    AX = mybir.AxisListType
    OP = mybir.AluOpType
    AF = mybir.ActivationFunctionType

    # Bacc (not plain Bass): its compile() runs generate_event_semaphores,
    # which splits multi-wait instructions to satisfy TRN2's 1-wait limit.
    nc = bacc.Bacc("TRN2", target_bir_lowering=False, num_devices=NCORES)

    xs = nc.dram_tensor("xs", [IMG, C, HP, WP], f32, kind="ExternalInput")
    wt = nc.dram_tensor("wt", [C, KPOS * C], f32, kind="ExternalInput")  # [cin, pos*C+cout]
    wn = nc.dram_tensor("wn", [C, KPOS * C], f32, kind="ExternalInput")  # [cout, k]
    gm = nc.dram_tensor("gamma", [C], f32, kind="ExternalInput")
    bt = nc.dram_tensor("beta", [C], f32, kind="ExternalInput")
    out = nc.dram_tensor("out", [IMG, C, H, W], f32, kind="ExternalOutput")

    from concourse.bass import _add_dep_helper

    with tile.TileContext(nc) as tc:
        with (
            tc.tile_pool(name="big", bufs=1) as big,
            tc.tile_pool(name="small", bufs=1) as small,
            tc.tile_pool(name="dram", bufs=1, space="DRAM") as dram,
            tc.tile_pool(name="psum", bufs=4, space="PSUM") as psum,
        ):
            # ---- warm-up collective: pays the communicator-init cost and
            # aligns the 8 cores while DMA/sign/conv run, so the real
            # AllReduce later hits its latency floor.
            warm_in = dram.tile([P, 1], f32, tag="warm_in", name="warm_in")
            warm_out = dram.tile([P, 1], f32, tag="warm_out", name="warm_out",
                                 addr_space="Shared")
            warm_cc = nc.gpsimd.collective_compute(
                "AllReduce", OP.add,
                replica_groups=[list(range(NCORES))],
                ins=[warm_in.opt()], outs=[warm_out.opt()],
            )

            # ---- critical path first: wt[0] + x[img0][cin0], then their signs
            wt_sb = []
            for t in range(CIN_T):
                wt_t = big.tile([P, KPOS * C], f32, tag=f"wt{t}", name=f"wt{t}")
                wt_sb.append(wt_t)
            xpad = [[None] * CIN_T for _ in range(IMG)]
            xsgn = [[None] * CIN_T for _ in range(IMG)]
            for img in range(IMG):
                for t in range(CIN_T):
                    xpad[img][t] = big.tile([P, HP, WP], f32, tag=f"xp{img}{t}",
                                            name=f"xp{img}{t}")
                    xsgn[img][t] = big.tile([P, HP, WP], bf16, tag=f"xg{img}{t}",
                                            name=f"xg{img}{t}")
            wsgn = []
            for t in range(CIN_T):
                sg = big.tile([P, KPOS * C], bf16, tag=f"wsgn{t}", name=f"wsgn{t}")
                wsgn.append(sg)

            # loads on two HWDGE rings: weights on SP, images on ACT
            nc.sync.dma_start(wt_sb[0], wt[0:P, :])
            nc.scalar.dma_start(xpad[0][0], xs[0, 0:P])
            nc.sync.dma_start(wt_sb[1], wt[P:2 * P, :])
            nc.scalar.dma_start(xpad[0][1], xs[0, P:2 * P])
            nc.scalar.dma_start(xpad[1][0], xs[1, 0:P])
            nc.scalar.dma_start(xpad[1][1], xs[1, P:2 * P])

            # signs, most-urgent first (conv group 0 needs wsgn0 + xg[0][0])
            nc.scalar.sign(wsgn[0], wt_sb[0])
            nc.scalar.sign(xsgn[0][0], xpad[0][0])  # sign(0)=0 keeps the padding
            nc.scalar.sign(wsgn[1], wt_sb[1])
            nc.scalar.sign(xsgn[0][1], xpad[0][1])
            nc.scalar.sign(xsgn[1][0], xpad[1][0])
            nc.scalar.sign(xsgn[1][1], xpad[1][1])

            # ---- non-critical loads via SWDGE: |w| scaling, gamma, beta ----
            wn_sb = []
            for t in range(CIN_T):
                wv = big.tile([P, KPOS * C], f32, tag=f"wn{t}", name=f"wn{t}")
                nc.gpsimd.dma_start(wv, wn[t * P:(t + 1) * P, :])
                wn_sb.append(wv)
            s_sb = small.tile([P, CT], f32, tag="s_sb", name="s_sb")
            for t in range(CT):
                nc.vector.tensor_reduce(
                    out=s_sb[:, t:t + 1], in_=wn_sb[t], axis=AX.X, op=OP.add,
                    apply_absolute_value=True,
                )
            nc.vector.tensor_scalar_mul(s_sb, s_sb, 1.0 / (KPOS * C))

            gm_sb = small.tile([P, CT], f32, tag="gm_sb", name="gm_sb")
            nc.gpsimd.dma_start(gm_sb, gm[:].rearrange("(t p) -> p t", p=P))
            bt_sb = small.tile([P, CT], f32, tag="bt_sb", name="bt_sb")
            nc.gpsimd.dma_start(bt_sb, bt[:].rearrange("(t p) -> p t", p=P))

            # ---- conv: per (cout_tile, img, l_half) accumulate 18 matmuls ----
            ysb = [[None] * CT for _ in range(IMG)]
            for img in range(IMG):
                for ct in range(CT):
                    ysb[img][ct] = big.tile([P, H * W], f32, tag=f"y{img}{ct}",
                                            name=f"y{img}{ct}")
            stats = [
                small.tile([P, IMG * 2, 6], f32, tag=f"st{ct}", name=f"st{ct}")
                for ct in range(CT)
            ]
            for ct in range(CT):
                for img in range(IMG):
                    for lh in range(2):
                        ps = psum.tile([P, N_HALF], f32, tag="ps", name="ps")
                        k = 0
                        for t in range(CIN_T):
                            for kh in range(3):
                                for kw in range(3):
                                    rhs = xsgn[img][t][
                                        :, lh * LH + kh: lh * LH + kh + LH, kw: kw + W
                                    ]
                                    pos = kh * 3 + kw
                                    lhsT = wsgn[t][:, pos * C + ct * P: pos * C + ct * P + P]
                                    nc.tensor.matmul(
                                        ps, lhsT, rhs, start=(k == 0), stop=(k == 17)
                                    )
                                    k += 1
                        yslice = ysb[img][ct][:, lh * N_HALF:(lh + 1) * N_HALF]
                        nc.scalar.copy(yslice, ps)  # evict raw conv ints to SBUF
                        nc.vector.bn_stats(stats[ct][:, img * 2 + lh, :], yslice)

            # ---- local stats -> per-channel (sum, sumsq) of y' ----
            sums = small.tile([P, CT, 2], f32, tag="sums", name="sums")
            for ct in range(CT):
                mv = small.tile([P, 2], f32, tag=f"mv{ct}", name=f"mv{ct}")
                nc.vector.bn_aggr(mv, stats[ct])
                nc.vector.tensor_scalar_mul(sums[:, ct, 0:1], mv[:, 0:1], NLOC)
                msq = small.tile([P, 1], f32, tag=f"msq{ct}", name=f"msq{ct}")
                nc.vector.tensor_tensor(msq, mv[:, 0:1], mv[:, 0:1], OP.mult)
                nc.vector.tensor_add(msq, msq, mv[:, 1:2])
                nc.vector.tensor_scalar_mul(sums[:, ct, 1:2], msq, NLOC)

            # ---- AllReduce the 2 KiB of partial sums across the 8 cores ----
            cc_in = dram.tile([P, CT * 2], f32, tag="cc_in", name="cc_in")
            cc_out = dram.tile([P, CT * 2], f32, tag="cc_out", name="cc_out",
                               addr_space="Shared")
            nc.sync.dma_start(cc_in[:, :], sums[:, :, :])
            real_cc = nc.gpsimd.collective_compute(
                "AllReduce", OP.add,
                replica_groups=[list(range(NCORES))],
                ins=[cc_in.opt()], outs=[cc_out.opt()],
            )
            # keep the warm-up collective ordered before the real one
            _add_dep_helper(real_cc.ins, warm_cc.ins, sync=True,
                            reason="collective warm-up ordering")
            tot = small.tile([P, CT, 2], f32, tag="tot", name="tot")
            nc.sync.dma_start(tot[:, :, :], cc_out[:, :])

            # ---- fold scaling + BN + gamma/beta into per-channel affine ----
            # mean' = S1/n ; var' = S2/n - mean'^2  (stats of raw conv y')
            # v = var' * s^2 + eps ; inv = rsqrt(v)
            # A = s * gamma * inv ; Bc = beta - mean' * A
            mp = small.tile([P, CT], f32, tag="mp", name="mp")
            nc.vector.tensor_scalar_mul(mp, tot[:, :, 0], 1.0 / NTOT)
            vv = small.tile([P, CT], f32, tag="vv", name="vv")
            nc.vector.tensor_scalar_mul(vv, tot[:, :, 1], 1.0 / NTOT)
            t2 = small.tile([P, CT], f32, tag="t2", name="t2")
            nc.vector.tensor_tensor(t2, mp, mp, OP.mult)
            nc.vector.tensor_tensor(vv, vv, t2, OP.subtract)      # var'
            nc.vector.tensor_tensor(t2, s_sb, s_sb, OP.mult)      # s^2
            nc.vector.tensor_tensor(vv, vv, t2, OP.mult)
            nc.vector.tensor_scalar_add(vv, vv, EPS)              # v
            sq = small.tile([P, CT], f32, tag="sq", name="sq")
            nc.scalar.sqrt(sq, vv)
            r0 = small.tile([P, CT], f32, tag="r0", name="r0")
            nc.vector.reciprocal(r0, sq)
            # one Newton step on rsqrt: r = r0 * (1.5 - 0.5 * v * r0^2)
            nc.vector.tensor_tensor(t2, vv, r0, OP.mult)
            nc.vector.tensor_tensor(t2, t2, r0, OP.mult)
            nc.vector.tensor_scalar(t2, t2, -0.5, 1.5, OP.mult, OP.add)
            nc.vector.tensor_tensor(r0, r0, t2, OP.mult)          # inv
            A_sb = small.tile([P, CT], f32, tag="A_sb", name="A_sb")
            nc.vector.tensor_tensor(A_sb, s_sb, gm_sb, OP.mult)
            nc.vector.tensor_tensor(A_sb, A_sb, r0, OP.mult)
            B_sb = small.tile([P, CT], f32, tag="B_sb", name="B_sb")
            nc.vector.tensor_tensor(B_sb, mp, A_sb, OP.mult)
            nc.vector.tensor_tensor(B_sb, bt_sb, B_sb, OP.subtract)

            # ---- apply affine + residual, write out ----
            for img in range(IMG):
                for ct in range(CT):
                    yo = big.tile([P, H, W], f32, tag=f"yo{img}{ct}", name=f"yo{img}{ct}")
                    nc.scalar.activation(
                        yo,
                        ysb[img][ct].rearrange("p (a b) -> p a b", b=W),
                        AF.Identity,
                        bias=B_sb[:, ct:ct + 1],
                        scale=A_sb[:, ct:ct + 1],
                    )
                    nc.vector.tensor_add(
                        yo, yo, xpad[img][ct][:, 1:H + 1, 1:W + 1]
                    )
                    nc.sync.dma_start(out[img, ct * P:(ct + 1) * P], yo)

    return nc


def _get_nc():
    if "nc" not in _NC_CACHE:
        nc = _build_nc()
        nc.finalize()  # Bacc defers register allocation to finalize()
        _NC_CACHE["nc"] = nc
    return _NC_CACHE["nc"]


def kernel(**inputs) -> np.ndarray:
    global LAST_RESULTS
    x = np.ascontiguousarray(np.asarray(inputs["x"], dtype=np.float32))
    w = np.asarray(inputs["weights"], dtype=np.float32)
    gamma = np.ascontiguousarray(np.asarray(inputs["gamma"], dtype=np.float32))
    beta = np.ascontiguousarray(np.asarray(inputs["beta"], dtype=np.float32))

    # host-side layout glue: zero-pad x to 30x32 rows, pre-transpose weights
    xp = np.zeros((B, C, HP, WP), np.float32)
    xp[:, :, 1:H + 1, 1:W + 1] = x
    wt = np.ascontiguousarray(
        w.transpose(1, 2, 3, 0).reshape(C, KPOS * C)   # [cin, (kh*3+kw)*C + cout]
    )
    wn = np.ascontiguousarray(w.reshape(C, KPOS * C))  # [cout, cin*9 + kh*3 + kw]

    nc = _get_nc()
    from concourse.bass_utils import run_bass_kernel_spmd

    in_maps = [
        {
            "xs": np.ascontiguousarray(xp[IMG * c: IMG * (c + 1)]),
            "wt": wt,
            "wn": wn,
            "gamma": gamma,
            "beta": beta,
        }
        for c in range(NCORES)
    ]
    res = run_bass_kernel_spmd(nc, in_maps, core_ids=list(range(NCORES)))
    LAST_RESULTS = res
    return np.concatenate([res.results[c]["out"] for c in range(NCORES)], axis=0)
